# revision 39
# baseline (speedup 1.0000x reference)
"""RNEA inverse dynamics v3: joint-in-partition layout, PE-matmul scans.

Layout per core: partition p = j*4 + blk (j in [0,32) joint, blk in [0,4)
sample-block), free dim = 2048 samples (4 chunks of F=512). Cumulative sums
along the chain are 128x128 block-triangular matmuls on the PE.

v3 changes vs v2 (91.6us -> 66.1us in TimelineSim):
- 3 inputs (q fp32/tf32, qd bf16, qdd bf16) instead of 4; per-joint angle
  offsets ride in per-partition [128,1] scalars of the magic-round
  tensor_scalar and the Sin bias AP; uM = uP + q/2pi on-chip (no uM scan).
- trig: kh=(u+c)+MAGIC (TS, 2x_2p), x1=(kh-MAGIC)-u (STT),
  sin(-2pi*x1 + 2pi*c) via ACT Sin with per-partition bias.
- damping folded into tau evacuation via scalar_tensor_tensor (mult,add).
- tau's 4 Wrirm products collapsed to 2 via A2 = Lx + Wc*Uy, B2 = Ly - Wc*Ux.
- rp folded into WcS/AcS scan weights (output-column scaling).
- engine balance per chunk: DVE 19 ops, ACT 13, Pool 8 (TT only -- gpsimd
  rejects TensorScalarPtr), PE 24 matmuls; PSUM-fed products read via ACT
  bf16 SBUF stages so Pool can run at all and DVE gets 2-byte operands.
- software-pipelined emission in waves front(w)/mid(w-1)/back(w-2) with
  PSUM rings sized so cross-wave buffer reuse only waits on early-dying
  stage copies (psA=4, psL=2, psB=2 banks of 8); per-chunk input/output
  DMA tiles; bf16 output.
"""
import numpy as np

B, N = 65536, 32
NCORES = 8
BC = B // NCORES            # 8192 samples per core
NBLK = 4                    # sample blocks -> partition p = j*NBLK + blk
FT = BC // NBLK             # 2048 free elems per partition
F = 512                     # chunk free size
CHUNKS = FT // F            # 4
TWO_PI = float(2 * np.pi)
INV2PI = float(1.0 / (2 * np.pi))
MAGIC = float(1.5 * 2**23)

_CACHE = {}


def _tf32(x):
    u = np.asarray(x, np.float32).view(np.uint32).astype(np.uint64)
    u = (u + 0x1000) & 0xFFFFE000
    return u.astype(np.uint32).view(np.float32)


def _build_nc():
    import concourse.bacc as bacc
    import concourse.bass as bass
    import concourse.mybir as mybir
    from concourse.tile import TileContext

    FP = mybir.dt.float32
    FR = mybir.dt.float32r
    BF = mybir.dt.bfloat16
    Op = mybir.AluOpType
    Act = mybir.ActivationFunctionType
    nc = bacc.Bacc()

    d_qf = nc.dram_tensor("qf", [N, BC], FR, kind="ExternalInput")
    d_qdb = nc.dram_tensor("qdb", [N, BC], BF, kind="ExternalInput")
    d_qddb = nc.dram_tensor("qddb", [N, BC], BF, kind="ExternalInput")
    d_wfr = nc.dram_tensor("wfr", [128, 128], FR, kind="ExternalInput")
    d_wbf = nc.dram_tensor("wbf", [128, 11 * 128], BF, kind="ExternalInput")
    d_cst = nc.dram_tensor("cst", [128, 16], FP, kind="ExternalInput")
    d_tau = nc.dram_tensor("tau", [N, BC], BF, kind="ExternalOutput")

    def full_view(d, dt_sz_elems):
        # whole [128, FT] partition-major view of a [N, BC] dram tensor
        return bass.AP(d, 0, [[FT, 128], [1, FT]])

    with TileContext(nc) as tc:
        with tc.tile_pool(name="wts", bufs=1) as wtp, \
             tc.tile_pool(name="fpw", bufs=40) as fpw, \
             tc.tile_pool(name="bfw", bufs=96) as bfw, \
             tc.tile_pool(name="psA", bufs=4, space="PSUM") as psA, \
             tc.tile_pool(name="psL", bufs=2, space="PSUM") as psL, \
             tc.tile_pool(name="psB", bufs=2, space="PSUM") as psB:

            # ---- resident weights / constants / inputs ----
            wfr = wtp.tile([128, 128], FR, tag="wfr", name="wfr")
            nc.sync.dma_start(out=wfr[:, :],
                              in_=bass.AP(d_wfr, 0, [[128, 128], [1, 128]]))

            qf_t, qdb_t, qddb_t = [], [], []
            wbf = cst = None
            for c in range(CHUNKS):
                for lst, nm, dt_, dram in ((qf_t, "qf", FR, d_qf),
                                           (qdb_t, "qdb", BF, d_qdb),
                                           (qddb_t, "qddb", BF, d_qddb)):
                    t_ = wtp.tile([128, F], dt_, tag=f"{nm}{c}",
                                  name=f"{nm}_{c}")
                    nc.sync.dma_start(
                        out=t_[:, :],
                        in_=bass.AP(dram, c * F, [[FT, 128], [1, F]]))
                    lst.append(t_)
                if c == 0:
                    # consts (tiny, gate trig) then the scan weights, after
                    # the chunk-0 inputs so neither delays the first scans
                    cst = wtp.tile([128, 16], FP, tag="cst", name="cst")
                    nc.sync.dma_start(
                        out=cst[:, :],
                        in_=bass.AP(d_cst, 0, [[16, 128], [1, 16]]))
                    wbf = wtp.tile([128, 11 * 128], BF, tag="wbf", name="wbf")
                    nc.sync.dma_start(
                        out=wbf[:, 0:384],
                        in_=bass.AP(d_wbf, 0, [[1408, 128], [1, 384]]))
                if c == 1:
                    # the eight mid/back weights can land a chunk later
                    nc.sync.dma_start(
                        out=wbf[:, 384:1408],
                        in_=bass.AP(d_wbf, 384, [[1408, 128], [1, 1024]]))

            (B_I, B_IN, B_ERP, B_M, B_MN, B_RM, B_RMN, B_RP, B_RPN,
             B_IBZ, B_DMP) = [wbf[:, i * 128:(i + 1) * 128] for i in range(11)]
            W_E2P = wfr[:, :]            # exclusive-scan / 2pi (fp32r)
            DMP = cst[:, 0:1]
            A_S, A_C, B_S, B_C = (cst[:, k:k + 1] for k in range(1, 5))
            BI_AS, BI_AC, BI_BS, BI_BC = (cst[:, k:k + 1] for k in range(5, 9))

            mm = nc.tensor.matmul
            V, S, G = nc.vector, nc.scalar, nc.gpsimd

            st = [dict() for _ in range(CHUNKS)]   # per-chunk tile refs

            def ftile(nm, c):
                return fpw.tile([128, F], FP, tag="fp", name=f"{nm}_{c}")

            def btile(nm, c):
                return bfw.tile([128, F], BF, tag="bf", name=f"{nm}_{c}")

            def ptile(pool, nm, c):
                return pool.tile([128, F], FP, tag=pool.name,
                                 name=f"{nm}_{c}")

            def emit_front(c):
                t = st[c]
                qf_c = qf_t[c].bitcast(FP)[:, :]
                qdb_c, qddb_c = qdb_t[c][:, :], qddb_t[c][:, :]
                uP = ptile(psA, "uP", c)
                mm(uP[:, :], W_E2P, qf_t[c][:, :])
                WcSr = ptile(psA, "WcSr", c)
                mm(WcSr[:, :], B_ERP, qdb_c)
                Ac = ptile(psA, "Ac", c)
                mm(Ac[:, :], B_I, qddb_c)
                AcSr = ptile(psA, "AcSr", c)
                mm(AcSr[:, :], B_ERP, qddb_c)
                Wc = ptile(psA, "Wc", c)
                mm(Wc[:, :], B_I, qdb_c)

                if c == 0:
                    # fill path: skip the ACT stage, read uP straight from
                    # PSUM (slower DVE ops but one less serial hop)
                    uPb = uP
                else:
                    uPb = ftile("uPb", c)
                    S.copy(uPb[:, :], uP[:, :])
                fMp = ftile("fMp", c)
                V.tensor_tensor(out=fMp[:, :], in0=qf_c, in1=uP[:, :],
                                op=Op.add)

                def trig(nm, u, off, bias):
                    kh = ftile("kh" + nm, c)
                    V.tensor_scalar(kh[:, :], u[:, :], off, MAGIC,
                                    Op.add, Op.add)
                    x1 = ftile("x1" + nm, c)
                    V.scalar_tensor_tensor(x1[:, :], kh[:, :], -MAGIC,
                                           u[:, :], Op.add, Op.subtract)
                    s = btile(nm, c)
                    S.activation(s[:, :], x1[:, :], Act.Sin,
                                 bias=bias, scale=-TWO_PI)
                    return s

                sinP = trig("sinP", uPb, A_S, BI_AS)
                cosP = trig("cosP", uPb, A_C, BI_AC)
                sinM = trig("sinM", fMp, B_S, BI_BS)
                cosM = trig("cosM", fMp, B_C, BI_BC)
                t.update(sinP=sinP, cosP=cosP, sinM=sinM, cosM=cosM)

                WcSrb = btile("WcSrb", c)
                S.copy(WcSrb[:, :], WcSr[:, :])
                AcSrb = btile("AcSrb", c)
                S.copy(AcSrb[:, :], AcSr[:, :])
                Wcb = btile("Wcb", c)
                S.copy(Wcb[:, :], Wc[:, :])
                W2 = btile("W2", c)
                V.tensor_tensor(out=W2[:, :], in0=Wcb[:, :], in1=Wcb[:, :],
                                op=Op.mult)
                Acb = btile("Acb", c)
                S.copy(Acb[:, :], Ac[:, :])
                t.update(W2=W2, Wcb=Wcb, Acb=Acb)

                tt1 = btile("tt1", c)
                V.tensor_tensor(out=tt1[:, :], in0=sinP[:, :],
                                in1=WcSrb[:, :], op=Op.mult)
                tt2 = btile("tt2", c)
                V.tensor_tensor(out=tt2[:, :], in0=cosP[:, :],
                                in1=WcSrb[:, :], op=Op.mult)
                a1 = btile("a1", c)
                V.tensor_tensor(out=a1[:, :], in0=sinP[:, :],
                                in1=AcSrb[:, :], op=Op.mult)
                a2 = btile("a2", c)
                V.tensor_tensor(out=a2[:, :], in0=cosP[:, :],
                                in1=AcSrb[:, :], op=Op.mult)
                t.update(a1=a1, a2=a2)

                Ux = ptile(psA, "Ux", c)
                mm(Ux[:, :], B_IN, tt1[:, :])
                Uy = ptile(psA, "Uy", c)
                mm(Uy[:, :], B_I, tt2[:, :])
                Uxb = btile("Uxb", c)
                S.copy(Uxb[:, :], Ux[:, :])
                Uyb = btile("Uyb", c)
                S.copy(Uyb[:, :], Uy[:, :])
                t.update(Uxb=Uxb, Uyb=Uyb)

            def emit_mid(c):
                t = st[c]
                qdb_c = qdb_t[c][:, :]
                E = G
                d1 = btile("d1", c)
                E.tensor_tensor(out=d1[:, :], in0=qdb_c, in1=t["Uyb"][:, :],
                                op=Op.mult)
                d2 = btile("d2", c)
                E.tensor_tensor(out=d2[:, :], in0=qdb_c, in1=t["Uxb"][:, :],
                                op=Op.mult)
                wux = btile("wux", c)
                E.tensor_tensor(out=wux[:, :], in0=t["Wcb"][:, :],
                                in1=t["Uxb"][:, :], op=Op.mult)
                wuy = btile("wuy", c)
                E.tensor_tensor(out=wuy[:, :], in0=t["Wcb"][:, :],
                                in1=t["Uyb"][:, :], op=Op.mult)
                m1 = btile("m1", c)
                E.tensor_tensor(out=m1[:, :], in0=t["Acb"][:, :],
                                in1=t["sinM"][:, :], op=Op.mult)
                m2 = btile("m2", c)
                E.tensor_tensor(out=m2[:, :], in0=t["Acb"][:, :],
                                in1=t["cosM"][:, :], op=Op.mult)
                w1 = btile("w1", c)
                E.tensor_tensor(out=w1[:, :], in0=t["W2"][:, :],
                                in1=t["cosM"][:, :], op=Op.mult)
                w2 = btile("w2", c)
                E.tensor_tensor(out=w2[:, :], in0=t["W2"][:, :],
                                in1=t["sinM"][:, :], op=Op.mult)
                t.update(m1=m1, m2=m2, wux=wux, wuy=wuy, w1=w1, w2=w2)

                Lx = ptile(psL, "Lx", c)
                mm(Lx[:, :], B_I, d1[:, :], start=True, stop=False)
                mm(Lx[:, :], B_IN, t["a1"][:, :], start=False, stop=True)
                Ly = ptile(psL, "Ly", c)
                mm(Ly[:, :], B_I, t["a2"][:, :], start=True, stop=False)
                mm(Ly[:, :], B_IN, d2[:, :], start=False, stop=True)
                Lxb = btile("Lxb", c)
                S.copy(Lxb[:, :], Lx[:, :])
                Lyb = btile("Lyb", c)
                S.copy(Lyb[:, :], Ly[:, :])
                t.update(Lxb=Lxb, Lyb=Lyb)

            def emit_back(c):
                t = st[c]
                qdb_c, qddb_c = qdb_t[c][:, :], qddb_t[c][:, :]
                A2c = btile("A2c", c)
                V.tensor_tensor(out=A2c[:, :], in0=t["Lxb"][:, :],
                                in1=t["wuy"][:, :], op=Op.add)
                B2c = btile("B2c", c)
                V.tensor_tensor(out=B2c[:, :], in0=t["Lyb"][:, :],
                                in1=t["wux"][:, :], op=Op.subtract)
                t4 = btile("t4", c)
                V.tensor_tensor(out=t4[:, :], in0=B2c[:, :],
                                in1=t["cosM"][:, :], op=Op.mult)
                t5 = btile("t5", c)
                V.tensor_tensor(out=t5[:, :], in0=A2c[:, :],
                                in1=t["sinM"][:, :], op=Op.mult)

                Fx = ptile(psB, "Fx", c)
                mm(Fx[:, :], B_M, t["Lxb"][:, :], start=True, stop=False)
                mm(Fx[:, :], B_RM, t["m1"][:, :], start=False, stop=False)
                mm(Fx[:, :], B_RM, t["w1"][:, :], start=False, stop=False)
                mm(Fx[:, :], B_MN, t["wuy"][:, :], start=False, stop=True)
                Fy = ptile(psB, "Fy", c)
                mm(Fy[:, :], B_M, t["Lyb"][:, :], start=True, stop=False)
                mm(Fy[:, :], B_RMN, t["m2"][:, :], start=False, stop=False)
                mm(Fy[:, :], B_RM, t["w2"][:, :], start=False, stop=False)
                mm(Fy[:, :], B_M, t["wux"][:, :], start=False, stop=True)

                e1 = btile("e1", c)
                V.tensor_tensor(out=e1[:, :], in0=t["cosP"][:, :],
                                in1=Fy[:, :], op=Op.mult)
                e1b = btile("e1b", c)
                V.tensor_tensor(out=e1b[:, :], in0=t["sinP"][:, :],
                                in1=Fx[:, :], op=Op.mult)

                tau = ptile(psB, "tau", c)
                mm(tau[:, :], B_IBZ, qddb_c, start=True, stop=False)
                mm(tau[:, :], B_RM, t4[:, :], start=False, stop=False)
                mm(tau[:, :], B_RMN, t5[:, :], start=False, stop=False)
                mm(tau[:, :], B_RP, e1[:, :], start=False, stop=False)
                mm(tau[:, :], B_RPN, e1b[:, :], start=False, stop=True)

                taub = bfw.tile([128, F], BF, tag="bf", name=f"taub_{c}")
                V.scalar_tensor_tensor(taub[:, :], qdb_c, DMP, tau[:, :],
                                       Op.mult, Op.add)
                nc.sync.dma_start(
                    out=bass.AP(d_tau, c * F, [[FT, 128], [1, F]]),
                    in_=taub[:, :])

            # software-pipelined emission: mid/back of older chunks first so
            # each engine stream interleaves three chunks
            for w in range(CHUNKS + 2):
                if w < CHUNKS:
                    emit_front(w)
                if 1 <= w <= CHUNKS:
                    emit_mid(w - 1)
                if 2 <= w:
                    emit_back(w - 2)

    nc.finalize()
    return nc


def _host_prep(q, qd, qdd_des, trans, mass, com, inertia, damping):
    import ml_dtypes
    px, py = trans[:, 0].astype(np.float64), trans[:, 1].astype(np.float64)
    mc = (mass[:, None] * com).astype(np.float64)
    mcx, mcy = mc[:, 0], mc[:, 1]

    def skew(v):
        x, y, z = v[..., 0], v[..., 1], v[..., 2]
        o = np.zeros_like(x)
        return np.stack([np.stack([o, -z, y], -1),
                         np.stack([z, o, -x], -1),
                         np.stack([-y, x, o], -1)], -2)
    Sk = skew(com.astype(np.float64))
    Ibar = inertia + (mass[:, None, None] * (Sk @ np.swapaxes(Sk, -1, -2))
                      ).astype(np.float32)
    ibzz = Ibar[:, 2, 2].astype(np.float64)

    rp = np.hypot(px, py)
    alpha = np.arctan2(py, px)
    rm = np.hypot(mcx, mcy)
    beta = np.arctan2(mcy, mcx)

    # transposed [N, B] inputs
    qf = _tf32(np.ascontiguousarray(q.T) * np.float32(INV2PI))
    qdb = np.ascontiguousarray(qd.T).astype(ml_dtypes.bfloat16)
    qddb = np.ascontiguousarray(qdd_des.T).astype(ml_dtypes.bfloat16)

    jj = np.arange(N)
    Ti = (jj[:, None] <= jj[None, :]).astype(np.float64)   # inclusive cumsum
    Te = (jj[:, None] < jj[None, :]).astype(np.float64)    # exclusive
    Tri = (jj[:, None] >= jj[None, :]).astype(np.float64)  # reverse inclusive
    Tre = (jj[:, None] > jj[None, :]).astype(np.float64)   # reverse exclusive

    def expand(T32):
        W = np.zeros((128, 128), np.float64)
        for blk in range(NBLK):
            W[blk::NBLK, blk::NBLK] = T32
        return W

    wfr = _tf32(expand(Te)).astype(np.float32)  # q is pre-scaled by 1/2pi

    Wibz32 = Ti @ np.diag(ibzz) @ Tri
    bf_list = [
        expand(Ti),                        # B_I
        expand(-Ti),                       # B_IN
        expand(Te * rp[None, :]),          # B_ERP (rp on out columns)
        expand(mass[:, None] * Tri),       # B_M
        expand(-mass[:, None] * Tri),      # B_MN
        expand(rm[:, None] * Tri),         # B_RM
        expand(-rm[:, None] * Tri),       # B_RMN
        expand(rp[:, None] * Tre),         # B_RP
        expand(-rp[:, None] * Tre),       # B_RPN
        expand(Wibz32),                    # B_IBZ
        expand(np.diag(damping.astype(np.float64))),  # B_DMP
    ]
    wbf = np.concatenate([w.astype(np.float32) for w in bf_list],
                         axis=1).astype(ml_dtypes.bfloat16)

    # per-partition constants [128, 16]
    cst = np.zeros((128, 16), np.float32)
    a2 = alpha * INV2PI
    b2 = beta * INV2PI
    offs = [a2, a2 + 0.25, b2, b2 + 0.25]
    for j in range(N):
        for blk in range(NBLK):
            p = j * NBLK + blk
            cst[p, 0] = damping[j]
            for k in range(4):
                cst[p, 1 + k] = offs[k][j]
                cst[p, 5 + k] = TWO_PI * offs[k][j]
    return qf, qdb, qddb, wfr, wbf, cst


def kernel(q, qd, qdd_des, trans, mass, com, inertia, damping):
    from concourse.bass_utils import run_bass_kernel_spmd

    q = np.asarray(q, np.float32)
    qd = np.asarray(qd, np.float32)
    qdd = np.asarray(qdd_des, np.float32)
    qf, qdb, qddb, wfr, wbf, cst = _host_prep(
        q, qd, qdd, np.asarray(trans), np.asarray(mass),
        np.asarray(com), np.asarray(inertia), np.asarray(damping))

    if "nc" not in _CACHE:
        _CACHE["nc"] = _build_nc()
    nc = _CACHE["nc"]

    in_maps = []
    for cix in range(NCORES):
        sl = slice(cix * BC, (cix + 1) * BC)
        in_maps.append({
            "qf": np.ascontiguousarray(qf[:, sl]),
            "qdb": np.ascontiguousarray(qdb[:, sl]),
            "qddb": np.ascontiguousarray(qddb[:, sl]),
            "wfr": wfr,
            "wbf": wbf,
            "cst": cst,
        })
    res = run_bass_kernel_spmd(nc, in_maps, list(range(NCORES)))
    return np.concatenate(
        [np.asarray(r["tau"], np.float32).T for r in res.results], 0)


# revision 40
# speedup vs baseline: 1.0062x; 1.0062x over previous
"""RNEA inverse dynamics v3: joint-in-partition layout, PE-matmul scans.

Layout per core: partition p = j*4 + blk (j in [0,32) joint, blk in [0,4)
sample-block), free dim = 2048 samples (4 chunks of F=512). Cumulative sums
along the chain are 128x128 block-triangular matmuls on the PE.

v3 changes vs v2 (91.6us -> 66.1us in TimelineSim):
- 3 inputs (q fp32/tf32, qd bf16, qdd bf16) instead of 4; per-joint angle
  offsets ride in per-partition [128,1] scalars of the magic-round
  tensor_scalar and the Sin bias AP; uM = uP + q/2pi on-chip (no uM scan).
- trig: kh=(u+c)+MAGIC (TS, 2x_2p), x1=(kh-MAGIC)-u (STT),
  sin(-2pi*x1 + 2pi*c) via ACT Sin with per-partition bias.
- damping folded into tau evacuation via scalar_tensor_tensor (mult,add).
- tau's 4 Wrirm products collapsed to 2 via A2 = Lx + Wc*Uy, B2 = Ly - Wc*Ux.
- rp folded into WcS/AcS scan weights (output-column scaling).
- engine balance per chunk: DVE 19 ops, ACT 13, Pool 8 (TT only -- gpsimd
  rejects TensorScalarPtr), PE 24 matmuls; PSUM-fed products read via ACT
  bf16 SBUF stages so Pool can run at all and DVE gets 2-byte operands.
- software-pipelined emission in waves front(w)/mid(w-1)/back(w-2) with
  PSUM rings sized so cross-wave buffer reuse only waits on early-dying
  stage copies (psA=4, psL=2, psB=2 banks of 8); per-chunk input/output
  DMA tiles; bf16 output.
"""
import numpy as np

B, N = 65536, 32
NCORES = 8
BC = B // NCORES            # 8192 samples per core
NBLK = 4                    # sample blocks -> partition p = j*NBLK + blk
FT = BC // NBLK             # 2048 free elems per partition
F = 512                     # chunk free size
CHUNKS = FT // F            # 4
TWO_PI = float(2 * np.pi)
INV2PI = float(1.0 / (2 * np.pi))
MAGIC = float(1.5 * 2**23)

_CACHE = {}


def _tf32(x):
    u = np.asarray(x, np.float32).view(np.uint32).astype(np.uint64)
    u = (u + 0x1000) & 0xFFFFE000
    return u.astype(np.uint32).view(np.float32)


def _build_nc():
    import concourse.bacc as bacc
    import concourse.bass as bass
    import concourse.mybir as mybir
    from concourse.tile import TileContext

    FP = mybir.dt.float32
    FR = mybir.dt.float32r
    BF = mybir.dt.bfloat16
    Op = mybir.AluOpType
    Act = mybir.ActivationFunctionType
    nc = bacc.Bacc()

    d_qf = nc.dram_tensor("qf", [N, BC], FR, kind="ExternalInput")
    d_qdb = nc.dram_tensor("qdb", [N, BC], BF, kind="ExternalInput")
    d_qddb = nc.dram_tensor("qddb", [N, BC], BF, kind="ExternalInput")
    d_wfr = nc.dram_tensor("wfr", [128, 128], FR, kind="ExternalInput")
    d_wbf = nc.dram_tensor("wbf", [128, 11 * 128], BF, kind="ExternalInput")
    d_cst = nc.dram_tensor("cst", [128, 16], FP, kind="ExternalInput")
    d_tau = nc.dram_tensor("tau", [N, BC], BF, kind="ExternalOutput")

    def full_view(d, dt_sz_elems):
        # whole [128, FT] partition-major view of a [N, BC] dram tensor
        return bass.AP(d, 0, [[FT, 128], [1, FT]])

    with TileContext(nc) as tc:
        with tc.tile_pool(name="wts", bufs=1) as wtp, \
             tc.tile_pool(name="fpw", bufs=40) as fpw, \
             tc.tile_pool(name="bfw", bufs=96) as bfw, \
             tc.tile_pool(name="psA", bufs=4, space="PSUM") as psA, \
             tc.tile_pool(name="psL", bufs=2, space="PSUM") as psL, \
             tc.tile_pool(name="psB", bufs=2, space="PSUM") as psB:

            # ---- resident weights / constants / inputs ----
            wfr = wtp.tile([128, 128], FR, tag="wfr", name="wfr")
            nc.sync.dma_start(out=wfr[:, :],
                              in_=bass.AP(d_wfr, 0, [[128, 128], [1, 128]]))

            qf_t, qdb_t, qddb_t = [], [], []
            wbf = cst = None
            for c in range(CHUNKS):
                for lst, nm, dt_, dram in ((qf_t, "qf", FR, d_qf),
                                           (qdb_t, "qdb", BF, d_qdb),
                                           (qddb_t, "qddb", BF, d_qddb)):
                    t_ = wtp.tile([128, F], dt_, tag=f"{nm}{c}",
                                  name=f"{nm}_{c}")
                    nc.sync.dma_start(
                        out=t_[:, :],
                        in_=bass.AP(dram, c * F, [[FT, 128], [1, F]]))
                    lst.append(t_)
                if c == 0:
                    # consts (tiny, gate trig) then the scan weights, after
                    # the chunk-0 inputs so neither delays the first scans
                    cst = wtp.tile([128, 16], FP, tag="cst", name="cst")
                    nc.sync.dma_start(
                        out=cst[:, :],
                        in_=bass.AP(d_cst, 0, [[16, 128], [1, 16]]))
                    wbf = wtp.tile([128, 11 * 128], BF, tag="wbf", name="wbf")
                    nc.sync.dma_start(
                        out=wbf[:, 0:384],
                        in_=bass.AP(d_wbf, 0, [[1408, 128], [1, 384]]))
                if c == 1:
                    # the eight mid/back weights can land a chunk later
                    nc.sync.dma_start(
                        out=wbf[:, 384:1408],
                        in_=bass.AP(d_wbf, 384, [[1408, 128], [1, 1024]]))

            (B_I, B_IN, B_ERP, B_M, B_MN, B_RM, B_RMN, B_RP, B_RPN,
             B_IBZ, B_DMP) = [wbf[:, i * 128:(i + 1) * 128] for i in range(11)]
            W_E2P = wfr[:, :]            # exclusive-scan / 2pi (fp32r)
            DMP = cst[:, 0:1]
            A_S, A_C, B_S, B_C = (cst[:, k:k + 1] for k in range(1, 5))
            BI_AS, BI_AC, BI_BS, BI_BC = (cst[:, k:k + 1] for k in range(5, 9))

            mm = nc.tensor.matmul
            V, S, G = nc.vector, nc.scalar, nc.gpsimd

            st = [dict() for _ in range(CHUNKS)]   # per-chunk tile refs

            def ftile(nm, c):
                return fpw.tile([128, F], FP, tag="fp", name=f"{nm}_{c}")

            def btile(nm, c):
                return bfw.tile([128, F], BF, tag="bf", name=f"{nm}_{c}")

            def ptile(pool, nm, c):
                return pool.tile([128, F], FP, tag=pool.name,
                                 name=f"{nm}_{c}")

            def emit_front(c):
                t = st[c]
                qf_c = qf_t[c].bitcast(FP)[:, :]
                qdb_c, qddb_c = qdb_t[c][:, :], qddb_t[c][:, :]
                uP = ptile(psA, "uP", c)
                mm(uP[:, :], W_E2P, qf_t[c][:, :])
                WcSr = ptile(psA, "WcSr", c)
                mm(WcSr[:, :], B_ERP, qdb_c)
                Ac = ptile(psA, "Ac", c)
                mm(Ac[:, :], B_I, qddb_c)
                AcSr = ptile(psA, "AcSr", c)
                mm(AcSr[:, :], B_ERP, qddb_c)
                Wc = ptile(psA, "Wc", c)
                mm(Wc[:, :], B_I, qdb_c)

                uPb = ftile("uPb", c)
                S.copy(uPb[:, :], uP[:, :])
                fMp = ftile("fMp", c)
                V.tensor_tensor(out=fMp[:, :], in0=qf_c, in1=uP[:, :],
                                op=Op.add)

                def trig(nm, u, off, bias):
                    kh = ftile("kh" + nm, c)
                    V.tensor_scalar(kh[:, :], u[:, :], off, MAGIC,
                                    Op.add, Op.add)
                    x1 = ftile("x1" + nm, c)
                    V.scalar_tensor_tensor(x1[:, :], kh[:, :], -MAGIC,
                                           u[:, :], Op.add, Op.subtract)
                    s = btile(nm, c)
                    S.activation(s[:, :], x1[:, :], Act.Sin,
                                 bias=bias, scale=-TWO_PI)
                    return s

                sinP = trig("sinP", uPb, A_S, BI_AS)
                cosP = trig("cosP", uPb, A_C, BI_AC)
                sinM = trig("sinM", fMp, B_S, BI_BS)
                cosM = trig("cosM", fMp, B_C, BI_BC)
                t.update(sinP=sinP, cosP=cosP, sinM=sinM, cosM=cosM)

                WcSrb = btile("WcSrb", c)
                S.copy(WcSrb[:, :], WcSr[:, :])
                AcSrb = btile("AcSrb", c)
                S.copy(AcSrb[:, :], AcSr[:, :])
                Wcb = btile("Wcb", c)
                S.copy(Wcb[:, :], Wc[:, :])
                W2 = btile("W2", c)
                V.tensor_tensor(out=W2[:, :], in0=Wcb[:, :], in1=Wcb[:, :],
                                op=Op.mult)
                Acb = btile("Acb", c)
                S.copy(Acb[:, :], Ac[:, :])
                t.update(W2=W2, Wcb=Wcb, Acb=Acb)

                tt1 = btile("tt1", c)
                V.tensor_tensor(out=tt1[:, :], in0=sinP[:, :],
                                in1=WcSrb[:, :], op=Op.mult)
                tt2 = btile("tt2", c)
                V.tensor_tensor(out=tt2[:, :], in0=cosP[:, :],
                                in1=WcSrb[:, :], op=Op.mult)
                a1 = btile("a1", c)
                V.tensor_tensor(out=a1[:, :], in0=sinP[:, :],
                                in1=AcSrb[:, :], op=Op.mult)
                a2 = btile("a2", c)
                V.tensor_tensor(out=a2[:, :], in0=cosP[:, :],
                                in1=AcSrb[:, :], op=Op.mult)
                t.update(a1=a1, a2=a2)

                Ux = ptile(psA, "Ux", c)
                mm(Ux[:, :], B_IN, tt1[:, :])
                Uy = ptile(psA, "Uy", c)
                mm(Uy[:, :], B_I, tt2[:, :])
                Uxb = btile("Uxb", c)
                S.copy(Uxb[:, :], Ux[:, :])
                Uyb = btile("Uyb", c)
                S.copy(Uyb[:, :], Uy[:, :])
                t.update(Uxb=Uxb, Uyb=Uyb)

            def emit_mid(c):
                t = st[c]
                qdb_c = qdb_t[c][:, :]
                E = G
                d1 = btile("d1", c)
                E.tensor_tensor(out=d1[:, :], in0=qdb_c, in1=t["Uyb"][:, :],
                                op=Op.mult)
                d2 = btile("d2", c)
                E.tensor_tensor(out=d2[:, :], in0=qdb_c, in1=t["Uxb"][:, :],
                                op=Op.mult)
                wux = btile("wux", c)
                E.tensor_tensor(out=wux[:, :], in0=t["Wcb"][:, :],
                                in1=t["Uxb"][:, :], op=Op.mult)
                wuy = btile("wuy", c)
                E.tensor_tensor(out=wuy[:, :], in0=t["Wcb"][:, :],
                                in1=t["Uyb"][:, :], op=Op.mult)
                m1 = btile("m1", c)
                E.tensor_tensor(out=m1[:, :], in0=t["Acb"][:, :],
                                in1=t["sinM"][:, :], op=Op.mult)
                m2 = btile("m2", c)
                E.tensor_tensor(out=m2[:, :], in0=t["Acb"][:, :],
                                in1=t["cosM"][:, :], op=Op.mult)
                w1 = btile("w1", c)
                E.tensor_tensor(out=w1[:, :], in0=t["W2"][:, :],
                                in1=t["cosM"][:, :], op=Op.mult)
                w2 = btile("w2", c)
                E.tensor_tensor(out=w2[:, :], in0=t["W2"][:, :],
                                in1=t["sinM"][:, :], op=Op.mult)
                t.update(m1=m1, m2=m2, wux=wux, wuy=wuy, w1=w1, w2=w2)

                Lx = ptile(psL, "Lx", c)
                mm(Lx[:, :], B_I, d1[:, :], start=True, stop=False)
                mm(Lx[:, :], B_IN, t["a1"][:, :], start=False, stop=True)
                Ly = ptile(psL, "Ly", c)
                mm(Ly[:, :], B_I, t["a2"][:, :], start=True, stop=False)
                mm(Ly[:, :], B_IN, d2[:, :], start=False, stop=True)
                Lxb = btile("Lxb", c)
                S.copy(Lxb[:, :], Lx[:, :])
                Lyb = btile("Lyb", c)
                S.copy(Lyb[:, :], Ly[:, :])
                t.update(Lxb=Lxb, Lyb=Lyb)

            def emit_back(c):
                t = st[c]
                qdb_c, qddb_c = qdb_t[c][:, :], qddb_t[c][:, :]
                A2c = btile("A2c", c)
                V.tensor_tensor(out=A2c[:, :], in0=t["Lxb"][:, :],
                                in1=t["wuy"][:, :], op=Op.add)
                B2c = btile("B2c", c)
                V.tensor_tensor(out=B2c[:, :], in0=t["Lyb"][:, :],
                                in1=t["wux"][:, :], op=Op.subtract)
                t4 = btile("t4", c)
                V.tensor_tensor(out=t4[:, :], in0=B2c[:, :],
                                in1=t["cosM"][:, :], op=Op.mult)
                t5 = btile("t5", c)
                V.tensor_tensor(out=t5[:, :], in0=A2c[:, :],
                                in1=t["sinM"][:, :], op=Op.mult)

                Fx = ptile(psB, "Fx", c)
                mm(Fx[:, :], B_M, t["Lxb"][:, :], start=True, stop=False)
                mm(Fx[:, :], B_RM, t["m1"][:, :], start=False, stop=False)
                mm(Fx[:, :], B_RM, t["w1"][:, :], start=False, stop=False)
                mm(Fx[:, :], B_MN, t["wuy"][:, :], start=False, stop=True)
                Fy = ptile(psB, "Fy", c)
                mm(Fy[:, :], B_M, t["Lyb"][:, :], start=True, stop=False)
                mm(Fy[:, :], B_RMN, t["m2"][:, :], start=False, stop=False)
                mm(Fy[:, :], B_RM, t["w2"][:, :], start=False, stop=False)
                mm(Fy[:, :], B_M, t["wux"][:, :], start=False, stop=True)

                e1 = btile("e1", c)
                V.tensor_tensor(out=e1[:, :], in0=t["cosP"][:, :],
                                in1=Fy[:, :], op=Op.mult)
                e1b = btile("e1b", c)
                V.tensor_tensor(out=e1b[:, :], in0=t["sinP"][:, :],
                                in1=Fx[:, :], op=Op.mult)

                tau = ptile(psB, "tau", c)
                mm(tau[:, :], B_IBZ, qddb_c, start=True, stop=False)
                mm(tau[:, :], B_RM, t4[:, :], start=False, stop=False)
                mm(tau[:, :], B_RMN, t5[:, :], start=False, stop=False)
                mm(tau[:, :], B_RP, e1[:, :], start=False, stop=False)
                mm(tau[:, :], B_RPN, e1b[:, :], start=False, stop=True)

                taub = bfw.tile([128, F], BF, tag="bf", name=f"taub_{c}")
                V.scalar_tensor_tensor(taub[:, :], qdb_c, DMP, tau[:, :],
                                       Op.mult, Op.add)
                nc.sync.dma_start(
                    out=bass.AP(d_tau, c * F, [[FT, 128], [1, F]]),
                    in_=taub[:, :])

            # software-pipelined emission: mid/back of older chunks first so
            # each engine stream interleaves three chunks
            for w in range(CHUNKS + 2):
                if w < CHUNKS:
                    emit_front(w)
                if 1 <= w <= CHUNKS:
                    emit_mid(w - 1)
                if 2 <= w:
                    emit_back(w - 2)

    nc.finalize()
    return nc


def _host_prep(q, qd, qdd_des, trans, mass, com, inertia, damping):
    import ml_dtypes
    px, py = trans[:, 0].astype(np.float64), trans[:, 1].astype(np.float64)
    mc = (mass[:, None] * com).astype(np.float64)
    mcx, mcy = mc[:, 0], mc[:, 1]

    def skew(v):
        x, y, z = v[..., 0], v[..., 1], v[..., 2]
        o = np.zeros_like(x)
        return np.stack([np.stack([o, -z, y], -1),
                         np.stack([z, o, -x], -1),
                         np.stack([-y, x, o], -1)], -2)
    Sk = skew(com.astype(np.float64))
    Ibar = inertia + (mass[:, None, None] * (Sk @ np.swapaxes(Sk, -1, -2))
                      ).astype(np.float32)
    ibzz = Ibar[:, 2, 2].astype(np.float64)

    rp = np.hypot(px, py)
    alpha = np.arctan2(py, px)
    rm = np.hypot(mcx, mcy)
    beta = np.arctan2(mcy, mcx)

    # transposed [N, B] inputs
    qf = _tf32(np.ascontiguousarray(q.T) * np.float32(INV2PI))
    qdb = np.ascontiguousarray(qd.T).astype(ml_dtypes.bfloat16)
    qddb = np.ascontiguousarray(qdd_des.T).astype(ml_dtypes.bfloat16)

    jj = np.arange(N)
    Ti = (jj[:, None] <= jj[None, :]).astype(np.float64)   # inclusive cumsum
    Te = (jj[:, None] < jj[None, :]).astype(np.float64)    # exclusive
    Tri = (jj[:, None] >= jj[None, :]).astype(np.float64)  # reverse inclusive
    Tre = (jj[:, None] > jj[None, :]).astype(np.float64)   # reverse exclusive

    def expand(T32):
        W = np.zeros((128, 128), np.float64)
        for blk in range(NBLK):
            W[blk::NBLK, blk::NBLK] = T32
        return W

    wfr = _tf32(expand(Te)).astype(np.float32)  # q is pre-scaled by 1/2pi

    Wibz32 = Ti @ np.diag(ibzz) @ Tri
    bf_list = [
        expand(Ti),                        # B_I
        expand(-Ti),                       # B_IN
        expand(Te * rp[None, :]),          # B_ERP (rp on out columns)
        expand(mass[:, None] * Tri),       # B_M
        expand(-mass[:, None] * Tri),      # B_MN
        expand(rm[:, None] * Tri),         # B_RM
        expand(-rm[:, None] * Tri),       # B_RMN
        expand(rp[:, None] * Tre),         # B_RP
        expand(-rp[:, None] * Tre),       # B_RPN
        expand(Wibz32),                    # B_IBZ
        expand(np.diag(damping.astype(np.float64))),  # B_DMP
    ]
    wbf = np.concatenate([w.astype(np.float32) for w in bf_list],
                         axis=1).astype(ml_dtypes.bfloat16)

    # per-partition constants [128, 16]
    cst = np.zeros((128, 16), np.float32)
    a2 = alpha * INV2PI
    b2 = beta * INV2PI
    offs = [a2, a2 + 0.25, b2, b2 + 0.25]
    for j in range(N):
        for blk in range(NBLK):
            p = j * NBLK + blk
            cst[p, 0] = damping[j]
            for k in range(4):
                cst[p, 1 + k] = offs[k][j]
                cst[p, 5 + k] = TWO_PI * offs[k][j]
    return qf, qdb, qddb, wfr, wbf, cst


def kernel(q, qd, qdd_des, trans, mass, com, inertia, damping):
    from concourse.bass_utils import run_bass_kernel_spmd

    q = np.asarray(q, np.float32)
    qd = np.asarray(qd, np.float32)
    qdd = np.asarray(qdd_des, np.float32)
    qf, qdb, qddb, wfr, wbf, cst = _host_prep(
        q, qd, qdd, np.asarray(trans), np.asarray(mass),
        np.asarray(com), np.asarray(inertia), np.asarray(damping))

    if "nc" not in _CACHE:
        _CACHE["nc"] = _build_nc()
    nc = _CACHE["nc"]

    in_maps = []
    for cix in range(NCORES):
        sl = slice(cix * BC, (cix + 1) * BC)
        in_maps.append({
            "qf": np.ascontiguousarray(qf[:, sl]),
            "qdb": np.ascontiguousarray(qdb[:, sl]),
            "qddb": np.ascontiguousarray(qddb[:, sl]),
            "wfr": wfr,
            "wbf": wbf,
            "cst": cst,
        })
    res = run_bass_kernel_spmd(nc, in_maps, list(range(NCORES)))
    return np.concatenate(
        [np.asarray(r["tau"], np.float32).T for r in res.results], 0)
